# revision 47
# baseline (speedup 1.0000x reference)
"""Trainium2 Bass kernel for the 3-layer AR GRU (nn_AR_RNN_GRU).

Strategy
--------
Data-parallel over batch across 8 NeuronCores (batch 8 per core) — batch
elements are fully independent, so sharding adds ZERO numerical error and
needs ZERO communication.  Each core runs the whole 64-warm + 63-AR
recurrence on its batch slice with all weights replicated (fp16 matmuls,
fp32 gate math — numerically identical to the single-core baseline).

The per-core program is latency-dominated, so the layout is built around
the tile-granular dependency tracker:

 * PSUM is split per layer into TWO single-buffered accumulation tiles:
   ZR = [z | r] and XH = [xh | hh].  sigma(z,r) therefore waits only the
   z/r matmuls, not the whole gate stream; each tile's only readers finish
   mid-chain, before the next step's gh stream reopens it (bufs=1 is safe
   and keeps all 6 tiles + 2 dense-readout banks within the 8 PSUM banks).
 * Gate math per layer-step (critical chain in *bold*):
     ACT  *zr = sigmoid([z|r])*            (one op; ACT runs nothing else
     DVE  *t1 = zr.r * XH.hh ; t1 += XH.xh*    between this and tanh)
     DVE  omz = 1 - zr.z ; zh = zr.z * hF      (off-chain, in tanh window)
     ACT  *hc = tanh(t1)*
     DVE  *q = omz*hc ; h16 = zh+q*  -> next matmul input (fp16)
     Pool hF = zh+q                   (fp32 state, off-path)
   Keeping omz/zh on DVE means q and h16 have no cross-engine waits
   except the tanh they truly depend on.
 * Streams are emitted in semaphore-gate order so the in-order PE queue
   never blocks ready work behind a waiting instruction.  The warm phase
   runs a 3-layer wavefront: iteration tt computes L0(tt), L1(tt-1),
   L2(tt-2) from tiles streamed in the previous iteration, so the three
   chains overlap on the engines; the AR phase is the same layer-major
   emission but consumes tiles within the step (strict serial chain).
 * The AR feedback folds dense+normalize into one matrix:
   gx0 = h2 @ (Wd @ (Wx0/std)) + beff (bias via an extra ones K-chunk);
   the real prediction h2 @ Wd streams off-path, staged in SBUF and
   DMA'd out in 8-step groups.
 * All DRAM images are per-partition contiguous; each weight loads with
   two large DMAs.
"""

import os
import sys

import numpy as np

try:
    import concourse.bass as bass  # noqa: F401
except ImportError:  # grading env fallback
    sys.path.insert(0, "/opt/trn_rl_repo")

import concourse.bass as bass
import concourse.mybir as mybir
import concourse.tile as tile
from concourse import bacc
from concourse.bass_utils import run_bass_kernel_spmd

F16 = np.float16

B = 64  # total batch
NCORES = 8
BC = B // NCORES  # per-core batch (8)
D = 512  # data dim
U = 768  # GRU units
G = 3 * U  # gate columns (z|r|h)
KU = U // 128  # 6 K-chunks for a 768-row operand
MD = D // 128  # 4 M-chunks of data columns

SEC = KU * BC  # 48 columns per gate section

T_IN = int(os.environ.get("GRU_TIN", "64"))
T_OUT = int(os.environ.get("GRU_TOUT", "64"))


def _prep_weight(w, bias=None):
    """[K, G] fp32 (+bias [G]) -> per-partition image [128, n_chunks*G] fp16
    (chunk k at cols [k*G:(k+1)*G]; bias as extra chunk, row 0)."""
    k, g = w.shape
    assert k % 128 == 0
    wp = w.reshape(k // 128, 128, g)
    if bias is not None and float(np.abs(bias).max()) > 0.0:
        bc = np.zeros((1, 128, g), np.float32)
        bc[0, 0, :] = bias
        wp = np.concatenate([wp, bc], axis=0)
    return np.ascontiguousarray(wp.transpose(1, 0, 2).reshape(128, -1)).astype(F16)


def _build(n_warm, n_ar, wx0_c, weff_c):
    nc = bacc.Bacc(num_devices=1, name="gru_ar_dp8")
    f32, f16 = mybir.dt.float32, mybir.dt.float16
    n_steps = n_warm + n_ar  # state steps (t = 0 .. n_steps-1)
    n_out = n_ar + 1

    # ---- DRAM I/O (all per-partition contiguous) ----
    wx0 = nc.dram_tensor("wx0", [128, wx0_c * G], f16, kind="ExternalInput")
    weff = nc.dram_tensor("weff", [128, weff_c * G], f16, kind="ExternalInput")
    wx_d = [None] + [
        nc.dram_tensor(f"wx{j}", [128, KU * G], f16, kind="ExternalInput")
        for j in (1, 2)
    ]
    wh_d = [
        nc.dram_tensor(f"wh{j}", [128, KU * G], f16, kind="ExternalInput")
        for j in range(3)
    ]
    wd_d = nc.dram_tensor("wd", [128, KU * D], f16, kind="ExternalInput")
    dbg_on = os.environ.get("GRU_DBG", "") == "1"
    dbg = (
        nc.dram_tensor("dbg", [128, 6 * SEC], f32, kind="ExternalOutput")
        if dbg_on
        else None
    )
    xt = nc.dram_tensor("xt", [128, n_warm * MD * BC], f16, kind="ExternalInput")
    h0f = nc.dram_tensor("h0f", [128, 3 * SEC], f32, kind="ExternalInput")
    ones = nc.dram_tensor("ones", [128, BC], f16, kind="ExternalInput")
    ident = nc.dram_tensor("ident", [128, 128], f16, kind="ExternalInput")
    out = nc.dram_tensor("out", [128, n_out * MD * BC], f32, kind="ExternalOutput")

    sig = mybir.ActivationFunctionType.Sigmoid
    tanh = mybir.ActivationFunctionType.Tanh
    alu = mybir.AluOpType

    with tile.TileContext(nc) as tc:
        with (
            tc.tile_pool(name="wpool", bufs=1) as wpool,
            tc.tile_pool(name="state", bufs=1) as spool,
            tc.tile_pool(name="gm", bufs=2) as gm,
            tc.tile_pool(name="prs", bufs=2) as prpool,
            tc.tile_pool(name="pr0", bufs=1, space="PSUM") as pr0,
            tc.tile_pool(name="pr1", bufs=1, space="PSUM") as pr1,
            tc.tile_pool(name="pr2", bufs=1, space="PSUM") as pr2,
            tc.tile_pool(name="px0", bufs=1, space="PSUM") as px0,
            tc.tile_pool(name="px1", bufs=1, space="PSUM") as px1,
            tc.tile_pool(name="px2", bufs=1, space="PSUM") as px2,
            tc.tile_pool(name="pp", bufs=2, space="PSUM") as ppool,
        ):
            prp = [pr0, pr1, pr2]
            pxp = [px0, px1, px2]

            # ---- constants + state ----
            ones_t = wpool.tile([128, BC], f16, tag="ones")
            nc.sync.dma_start(ones_t[:], ones[:])
            ident_t = wpool.tile([128, 128], f16, tag="ident")
            nc.sync.dma_start(ident_t[:], ident[:])
            hF = []
            hT = []  # rings of 2 per layer
            for j in range(3):
                f = spool.tile([128, SEC], f32, tag=f"hF{j}")
                nc.sync.dma_start(f[:], h0f[:, j * SEC : (j + 1) * SEC])
                hF.append(f)
                ring = []
                for p in range(2):
                    t = spool.tile([128, SEC], f16, tag=f"hT{j}_{p}")
                    ring.append(t)
                hT.append(ring)
            for j in range(3):
                nc.vector.tensor_copy(hT[j][1][:], hF[j][:])  # h(-1) parity 1

            # ---- weights: 2 large DMAs each, first-use order ----
            def load_w(dram, ncols, tag, parts=2):
                t = wpool.tile([128, ncols], f16, tag=tag)
                step = (ncols + parts - 1) // parts
                for c in range(0, ncols, step):
                    e = min(c + step, ncols)
                    nc.sync.dma_start(t[:, c:e], dram[:, c:e])
                return t

            wh_t = [load_w(wh_d[0], KU * G, "wh0")]
            xall = wpool.tile([128, n_warm * MD * BC], f16, tag="xall")
            nc.sync.dma_start(xall[:], xt[:])
            nshare = max(wx0_c, weff_c)
            wshare = wpool.tile([128, nshare * G], f16, tag="wx0weff")
            nc.sync.dma_start(wshare[:, 0 : wx0_c * G], wx0[:, 0 : wx0_c * G])
            if weff_c > wx0_c:  # weff tail never collides with warm reads
                nc.sync.dma_start(
                    wshare[:, wx0_c * G : weff_c * G],
                    weff[:, wx0_c * G : weff_c * G],
                )
            wh_t.append(load_w(wh_d[1], KU * G, "wh1"))
            wx_t = [wshare, load_w(wx_d[1], KU * G, "wx1")]
            wh_t.append(load_w(wh_d[2], KU * G, "wh2"))
            wx_t.append(load_w(wx_d[2], KU * G, "wx2"))
            wd_t = load_w(wd_d, KU * D, "wd", parts=1)

            # ---- stream emitters ----
            # ZR tile: [z | r]; XH tile: [xh | hh] (closed by the id-add).
            R_cur = {}
            ZXH_cur = {}
            first = {}

            def new_tiles(j):
                R_cur[j] = prp[j].tile(
                    [128, 2 * SEC], f32, tag="zr", name=f"zr{j}"
                )
                ZXH_cur[j] = pxp[j].tile(
                    [128, 2 * SEC], f32, tag="xh", name=f"xh{j}"
                )
                first[j] = {"r": True, "zxh": True}

            def _mm(j, tile_kind, P, col, w_t, wcol, rhs, stop):
                nc.tensor.matmul(
                    P[:, col : col + BC],
                    w_t[:, wcol : wcol + 128],
                    rhs,
                    start=first[j][tile_kind],
                    stop=stop,
                    skip_group_check=True,
                )
                first[j][tile_kind] = False

            def gh_zr(j, t):
                """gh r then z sections from h_j(t-1) into the [z|r] tile."""
                src = hT[j][(t - 1) % 2]
                for m in range(6, 12):
                    c = m % 6
                    for k in range(KU):
                        _mm(j, "r", R_cur[j], SEC + c * BC,
                            wh_t[j], k * G + m * 128,
                            src[:, k * BC : (k + 1) * BC], False)
                for m in range(6):
                    for k in range(KU):
                        _mm(j, "r", R_cur[j], m * BC,
                            wh_t[j], k * G + m * 128,
                            src[:, k * BC : (k + 1) * BC], False)

            def gh_hh(j, t):
                src = hT[j][(t - 1) % 2]
                for m in range(12, 18):
                    c = m % 6
                    for k in range(KU):
                        _mm(j, "zxh", ZXH_cur[j], SEC + c * BC,
                            wh_t[j], k * G + m * 128,
                            src[:, k * BC : (k + 1) * BC], False)

            def gx_zr(j, w_t, kc, rhs_fn, closer=True):
                """gx r then z sections; the last z matmul closes [z|r]."""
                n = 0
                for m in list(range(6, 12)) + list(range(6)):
                    off = SEC if m >= 6 else 0
                    c = m % 6
                    for k in range(kc):
                        n += 1
                        _mm(j, "r", R_cur[j], off + c * BC,
                            w_t, k * G + m * 128, rhs_fn(k),
                            closer and n == 12 * kc)

            def gx_xh(j, w_t, kc, rhs_fn):
                """gx candidate section; the t1 id-accumulate closes [xh|hh]."""
                for m in range(12, 18):
                    c = m % 6
                    for k in range(kc):
                        _mm(j, "zxh", ZXH_cur[j], c * BC,
                            w_t, k * G + m * 128, rhs_fn(k), False)

            def rhs_x(s):
                def rhs(k, _s=s):
                    return xall[:, (_s * MD + k) * BC : (_s * MD + k + 1) * BC]
                return rhs

            def rhs_h(j, t):
                def rhs(k, _t=t):
                    if k >= KU:
                        return ones_t[:, 0:BC]
                    return hT[j][_t % 2][:, k * BC : (k + 1) * BC]
                return rhs

            # ---- gate math ----
            def math_p1(j):
                """sigma(r) only: the chain head."""
                rs = gm.tile([128, SEC], f32, tag=f"rs{j}")
                nc.scalar.activation(rs[:], R_cur[j][:, SEC : 2 * SEC], sig)
                return rs

            def math_p2(j, rs):
                """t1 = r*hh in fp16, accumulated into the xh PSUM region by
                an identity matmul — no DVE add, and this closes [xh|hh]."""
                P = ZXH_cur[j]
                t1 = gm.tile([128, SEC], f16, tag=f"t1{j}")
                nc.vector.tensor_mul(t1[:], rs[:], P[:, SEC : 2 * SEC])
                nc.tensor.matmul(
                    P[:, 0:SEC], ident_t[:], t1[:],
                    start=False, stop=True, skip_group_check=True,
                )

            def math_p3(j, ar=False):
                """sigma(z) + products (DVE, overlapped with the id-add/tanh)."""
                zs = gm.tile([128, SEC], f32, tag=f"zs{j}")
                nc.scalar.activation(zs[:], R_cur[j][:, 0:SEC], sig)
                omz = gm.tile([128, SEC], f32, tag=f"omz{j}")
                nc.vector.tensor_scalar(
                    omz[:], zs[:], -1.0, 1.0, alu.mult, alu.add
                )
                zh = gm.tile([128, SEC], f32, tag=f"zh{j}")
                nc.vector.tensor_mul(zh[:], zs[:], hF[j][:])
                zh16 = None
                if ar:
                    zh16 = gm.tile([128, SEC], f16, tag=f"zh16{j}")
                    nc.vector.tensor_copy(zh16[:], zh[:])
                return omz, zh, zh16

            def math_p4a(j):
                """tanh straight from PSUM; q in fp16 (a matmul rhs in AR)."""
                P = ZXH_cur[j]
                hc = gm.tile([128, SEC], f32, tag=f"hc{j}")
                nc.scalar.activation(hc[:], P[:, 0:SEC], tanh)
                q = gm.tile([128, SEC], f16, tag=f"q{j}")
                nc.vector.tensor_mul(q[:], omz_of[j][:], hc[:])
                return q

            omz_of = {}

            def math_p4b(j, t, zh, q):
                nc.vector.tensor_add(hT[j][t % 2][:], zh[:], q[:])
                nc.gpsimd.tensor_add(hF[j][:], zh[:], q[:])

            def math_full(j, t):
                rs = math_p1(j)
                math_p2(j, rs)
                omz, zh, _ = math_p3(j)
                omz_of[j] = omz
                q = math_p4a(j)
                math_p4b(j, t, zh, q)

            # ---- dense readout (off the critical chain) ----
            prs_state = {}

            def dense_mm(t):
                Pp = ppool.tile([128, MD * BC], f32, tag="pred", name="pred")
                src = hT[2][(t - 1) % 2]
                n = 0
                for k in range(KU):
                    for m in range(MD):
                        n += 1
                        nc.tensor.matmul(
                            Pp[:, m * BC : (m + 1) * BC],
                            wd_t[:, k * D + m * 128 : k * D + (m + 1) * 128],
                            src[:, k * BC : (k + 1) * BC],
                            start=n == 1,
                            stop=n == KU * MD,
                            skip_group_check=True,
                        )
                prs_state["Pp"] = Pp

            def dense_out(t):
                s = t - n_warm
                Pp = prs_state.pop("Pp")
                sl = s % 8
                if sl == 0:
                    prs_state["buf"] = prpool.tile(
                        [128, 8 * MD * BC], f32, tag="prs", name="prs"
                    )
                prs = prs_state["buf"]
                nc.vector.tensor_copy(
                    prs[:, sl * MD * BC : (sl + 1) * MD * BC], Pp[:]
                )
                if sl == 7 or s == n_ar:
                    grp = s // 8
                    nc.sync.dma_start(
                        out[:, grp * 8 * MD * BC : (grp * 8 + sl + 1) * MD * BC],
                        prs[:, 0 : (sl + 1) * MD * BC],
                    )

            # ================= WARM phase: pipelined 3-layer wavefront ======
            # iteration tt runs maths for L0(tt), L1(tt-1), L2(tt-2) on tiles
            # streamed during iteration tt-1; streams for the next maths are
            # emitted right after the h-state they read is produced.
            for j in range(3):
                new_tiles(j)
            # prologue: tiles for (j, 0) from the initial states / x(0)
            gh_zr(0, 0)
            gh_hh(0, 0)
            gx_zr(0, wshare, wx0_c, rhs_x(0))
            gx_xh(0, wshare, wx0_c, rhs_x(0))
            gh_zr(1, 0)
            gh_hh(1, 0)
            gh_zr(2, 0)
            gh_hh(2, 0)

            for tt in range(n_warm + 2):
                for j in range(3):
                    s = tt - j
                    if not (0 <= s < n_warm):
                        continue
                    math_full(j, s)
                    nxt = s + 1
                    if nxt < n_warm:
                        # streams for (j, nxt): gx reads h_{j-1}(nxt), which
                        # block j-1 of THIS iteration produced (or x); gh
                        # reads h_j(s), just produced above.
                        new_tiles(j)
                        if j == 0:
                            rx = rhs_x(nxt)
                            gx_zr(0, wshare, wx0_c, rx)
                            gh_zr(0, nxt)
                            gx_xh(0, wshare, wx0_c, rx)
                            gh_hh(0, nxt)
                        else:
                            rh = rhs_h(j - 1, nxt)
                            gx_zr(j, wx_t[j], KU, rh)
                            gh_zr(j, nxt)
                            gx_xh(j, wx_t[j], KU, rh)
                            gh_hh(j, nxt)
                    if s == 0 and j < 2:
                        # bootstrap: close tiles(j+1, 0) — its gh half was
                        # emitted in the prologue, gx needs h_j(0) from above
                        rh = rhs_h(j, 0)
                        gx_zr(j + 1, wx_t[j + 1], KU, rh)
                        gx_xh(j + 1, wx_t[j + 1], KU, rh)
                    if j == 0 and s == n_warm - 1:
                        # swap Weff over the wx0 chunks (after last warm use)
                        half = (wx0_c * G) // 2
                        nc.sync.dma_start(wshare[:, 0:half], weff[:, 0:half])
                        nc.sync.dma_start(
                            wshare[:, half : wx0_c * G], weff[:, half : wx0_c * G]
                        )

            # ================= AR phase: strictly serial =================
            # Layer handoffs stream W.(zh16) as soon as the z-products exist
            # (pass A, with the bias ones-chunk) and W.(q16) right after the
            # tanh (pass B) — the next layer never waits for the h16 add.
            def rhs_zh16(zz):
                def rhs(k, _z=zz):
                    if k >= KU:
                        return ones_t[:, 0:BC]
                    return _z[:, k * BC : (k + 1) * BC]
                return rhs

            def rhs_q16(qq):
                def rhs(k, _q=qq):
                    return _q[:, k * BC : (k + 1) * BC]
                return rhs

            for t in range(n_warm, n_steps):
                if t == n_warm:
                    # boundary: classic streams from the warm states
                    for j in range(3):
                        new_tiles(j)
                    gh_zr(0, t)
                    gh_hh(0, t)
                    gh_zr(1, t)
                    gh_hh(1, t)
                    gx_zr(0, wshare, weff_c, rhs_h(2, t - 1))
                    gx_xh(0, wshare, weff_c, rhs_h(2, t - 1))
                dense_mm(t)
                if t == n_warm:
                    gh_zr(2, t)
                    gh_hh(2, t)
                for j in range(3):
                    rs = math_p1(j)
                    math_p2(j, rs)
                    omz, zh, zh16 = math_p3(j, ar=True)
                    omz_of[j] = omz
                    # pass A of the next consumer's gx
                    if j < 2:
                        gx_zr(j + 1, wx_t[j + 1], KU, rhs_zh16(zh16),
                              closer=False)
                        gx_xh(j + 1, wx_t[j + 1], KU, rhs_zh16(zh16))
                    elif t + 1 < n_steps:
                        gx_zr(0, wshare, weff_c, rhs_zh16(zh16), closer=False)
                        gx_xh(0, wshare, weff_c, rhs_zh16(zh16))
                    q = math_p4a(j)
                    # pass B closes the next ZR tile; XH closes via its id-add
                    if j < 2:
                        gx_zr(j + 1, wx_t[j + 1], KU, rhs_q16(q))
                        gx_xh(j + 1, wx_t[j + 1], KU, rhs_q16(q))
                    elif t + 1 < n_steps:
                        gx_zr(0, wshare, KU, rhs_q16(q))
                        gx_xh(0, wshare, KU, rhs_q16(q))
                    math_p4b(j, t, zh, q)
                    if j == 0:
                        dense_out(t)
                    if t + 1 < n_steps:
                        if j < 2:
                            new_tiles(j)
                            gh_zr(j, t + 1)
                            gh_hh(j, t + 1)
                # gh2(t+1) + its tile rotation at the next step top
                if t + 1 < n_steps:
                    new_tiles(2)
                    gh_zr(2, t + 1)
                    gh_hh(2, t + 1)

            # final prediction from h2(n_steps-1)
            dense_mm(n_steps)
            dense_out(n_steps)
    nc.finalize()
    return nc


def kernel(**inputs):
    x = np.asarray(inputs["inputs"], np.float32)
    n_warm, n_ar = T_IN, T_OUT - 1
    x = x[:, :n_warm, :]

    mean = np.asarray(inputs["mean"], np.float32)[0]
    std = np.asarray(inputs["std"], np.float32)[0]
    wd_m = np.asarray(inputs["Wd"], np.float32)
    bd = np.asarray(inputs["bd"], np.float32)
    w1 = np.asarray(inputs["Wx0"], np.float32) / std[:, None]
    weff_m = wd_m @ w1
    beff = (bd - mean) @ w1 + np.asarray(inputs["bi0"], np.float32)

    wx0_a = _prep_weight(np.asarray(inputs["Wx0"], np.float32))
    wx0_c = wx0_a.shape[1] // G
    weff_a = _prep_weight(weff_m, beff)
    weff_c = weff_a.shape[1] // G
    wx_a = {
        j: _prep_weight(np.asarray(inputs[f"Wx{j}"], np.float32)) for j in (1, 2)
    }
    wh_a = {
        j: _prep_weight(np.asarray(inputs[f"Wh{j}"], np.float32)) for j in range(3)
    }
    wd_a = _prep_weight(wd_m)  # bd is zero

    # warm inputs: xt[p, t*MD*BC + k*BC + bi] = x[core*BC+bi, t, k*128+p]
    xt_cores = []
    for core in range(NCORES):
        xs = x[core * BC : (core + 1) * BC]  # [BC, T, D]
        xr = xs.reshape(BC, n_warm, MD, 128).transpose(3, 1, 2, 0)
        xt_cores.append(
            np.ascontiguousarray(xr.reshape(128, n_warm * MD * BC)).astype(F16)
        )

    # initial state: h0f[p, j*SEC + k*BC + bi] = h0_j[k*128+p]
    h0_parts = []
    for j in range(3):
        h = np.asarray(inputs[f"h0_{j}"], np.float32).reshape(KU, 128)
        h0_parts.append(np.repeat(h.transpose(1, 0)[:, :, None], BC, axis=2))
    h0f_a = np.concatenate(h0_parts, axis=1).reshape(128, 3 * SEC)
    h0f_a = np.ascontiguousarray(h0f_a).astype(np.float32)

    ones_a = np.zeros((128, BC), np.float32)
    ones_a[0, :] = 1.0
    ones_a = ones_a.astype(F16)
    ident_a = np.eye(128, dtype=F16)

    nc = _build(n_warm, n_ar, wx0_c, weff_c)
    in_maps = []
    for core in range(NCORES):
        in_maps.append(
            {
                "wx0": wx0_a,
                "weff": weff_a,
                "wx1": wx_a[1],
                "wx2": wx_a[2],
                "wh0": wh_a[0],
                "wh1": wh_a[1],
                "wh2": wh_a[2],
                "wd": wd_a,
                "xt": xt_cores[core],
                "h0f": h0f_a,
                "ones": ones_a,
                "ident": ident_a,
            }
        )
    res = run_bass_kernel_spmd(
        nc,
        in_maps,
        core_ids=list(range(NCORES)),
        trace=os.environ.get("GRU_TRACE", "") == "1",
    )
    kernel._last = res
    kernel._last_nc = nc

    n_out = n_ar + 1
    full = np.empty((B, n_out, D), np.float32)
    for core in range(NCORES):
        o = np.asarray(res.results[core]["out"], np.float32)
        o = o.reshape(128, n_out, MD, BC)
        full[core * BC : (core + 1) * BC] = o.transpose(3, 1, 2, 0).reshape(
            BC, n_out, D
        )
    return full


if __name__ == "__main__":
    print("smoke build only")


# revision 48
# speedup vs baseline: 1.0466x; 1.0466x over previous
"""Trainium2 Bass kernel for the 3-layer AR GRU (nn_AR_RNN_GRU).

Strategy
--------
Data-parallel over batch across 8 NeuronCores (batch 8 per core) — batch
elements are fully independent, so sharding adds ZERO numerical error and
needs ZERO communication.  Each core runs the whole 64-warm + 63-AR
recurrence on its batch slice with all weights replicated (fp16 matmuls,
fp32 gate math — numerically identical to the single-core baseline).

The per-core program is latency-dominated, so the layout is built around
the tile-granular dependency tracker:

 * PSUM is split per layer into TWO single-buffered accumulation tiles:
   ZR = [z | r] and XH = [xh | hh].  sigma(z,r) therefore waits only the
   z/r matmuls, not the whole gate stream; each tile's only readers finish
   mid-chain, before the next step's gh stream reopens it (bufs=1 is safe
   and keeps all 6 tiles + 2 dense-readout banks within the 8 PSUM banks).
 * Gate math per layer-step (critical chain in *bold*):
     ACT  *zr = sigmoid([z|r])*            (one op; ACT runs nothing else
     DVE  *t1 = zr.r * XH.hh ; t1 += XH.xh*    between this and tanh)
     DVE  omz = 1 - zr.z ; zh = zr.z * hF      (off-chain, in tanh window)
     ACT  *hc = tanh(t1)*
     DVE  *q = omz*hc ; h16 = zh+q*  -> next matmul input (fp16)
     Pool hF = zh+q                   (fp32 state, off-path)
   Keeping omz/zh on DVE means q and h16 have no cross-engine waits
   except the tanh they truly depend on.
 * Streams are emitted in semaphore-gate order so the in-order PE queue
   never blocks ready work behind a waiting instruction.  The warm phase
   runs a 3-layer wavefront: iteration tt computes L0(tt), L1(tt-1),
   L2(tt-2) from tiles streamed in the previous iteration, so the three
   chains overlap on the engines; the AR phase is the same layer-major
   emission but consumes tiles within the step (strict serial chain).
 * The AR feedback folds dense+normalize into one matrix:
   gx0 = h2 @ (Wd @ (Wx0/std)) + beff (bias via an extra ones K-chunk);
   the real prediction h2 @ Wd streams off-path, staged in SBUF and
   DMA'd out in 8-step groups.
 * All DRAM images are per-partition contiguous; each weight loads with
   two large DMAs.
"""

import os
import sys

import numpy as np

try:
    import concourse.bass as bass  # noqa: F401
except ImportError:  # grading env fallback
    sys.path.insert(0, "/opt/trn_rl_repo")

import concourse.bass as bass
import concourse.mybir as mybir
import concourse.tile as tile
from concourse import bacc
from concourse.bass_utils import run_bass_kernel_spmd

F16 = np.float16

B = 64  # total batch
NCORES = 8
BC = B // NCORES  # per-core batch (8)
D = 512  # data dim
U = 768  # GRU units
G = 3 * U  # gate columns (z|r|h)
KU = U // 128  # 6 K-chunks for a 768-row operand
MD = D // 128  # 4 M-chunks of data columns

SEC = KU * BC  # 48 columns per gate section

T_IN = int(os.environ.get("GRU_TIN", "64"))
T_OUT = int(os.environ.get("GRU_TOUT", "64"))


def _prep_weight(w, bias=None):
    """[K, G] fp32 (+bias [G]) -> per-partition image [128, n_chunks*G] fp16
    (chunk k at cols [k*G:(k+1)*G]; bias as extra chunk, row 0)."""
    k, g = w.shape
    assert k % 128 == 0
    wp = w.reshape(k // 128, 128, g)
    if bias is not None and float(np.abs(bias).max()) > 0.0:
        bc = np.zeros((1, 128, g), np.float32)
        bc[0, 0, :] = bias
        wp = np.concatenate([wp, bc], axis=0)
    return np.ascontiguousarray(wp.transpose(1, 0, 2).reshape(128, -1)).astype(F16)


def _build(n_warm, n_ar, wx0_c, weff_c):
    nc = bacc.Bacc(num_devices=1, name="gru_ar_dp8")
    f32, f16 = mybir.dt.float32, mybir.dt.float16
    n_steps = n_warm + n_ar  # state steps (t = 0 .. n_steps-1)
    n_out = n_ar + 1

    # ---- DRAM I/O (all per-partition contiguous) ----
    wx0 = nc.dram_tensor("wx0", [128, wx0_c * G], f16, kind="ExternalInput")
    weff = nc.dram_tensor("weff", [128, weff_c * G], f16, kind="ExternalInput")
    wx_d = [None] + [
        nc.dram_tensor(f"wx{j}", [128, KU * G], f16, kind="ExternalInput")
        for j in (1, 2)
    ]
    wh_d = [
        nc.dram_tensor(f"wh{j}", [128, KU * G], f16, kind="ExternalInput")
        for j in range(3)
    ]
    wd_d = nc.dram_tensor("wd", [128, KU * D], f16, kind="ExternalInput")
    dbg_on = os.environ.get("GRU_DBG", "") == "1"
    dbg = (
        nc.dram_tensor("dbg", [128, 6 * SEC], f32, kind="ExternalOutput")
        if dbg_on
        else None
    )
    xt = nc.dram_tensor("xt", [128, n_warm * MD * BC], f16, kind="ExternalInput")
    h0f = nc.dram_tensor("h0f", [128, 3 * SEC], f32, kind="ExternalInput")
    ones = nc.dram_tensor("ones", [128, BC], f16, kind="ExternalInput")
    ident = nc.dram_tensor("ident", [128, 128], f16, kind="ExternalInput")
    out = nc.dram_tensor("out", [128, n_out * MD * BC], f32, kind="ExternalOutput")

    sig = mybir.ActivationFunctionType.Sigmoid
    tanh = mybir.ActivationFunctionType.Tanh
    alu = mybir.AluOpType

    with tile.TileContext(nc) as tc:
        with (
            tc.tile_pool(name="wpool", bufs=1) as wpool,
            tc.tile_pool(name="state", bufs=1) as spool,
            tc.tile_pool(name="gm", bufs=2) as gm,
            tc.tile_pool(name="prs", bufs=2) as prpool,
            tc.tile_pool(name="pr0", bufs=1, space="PSUM") as pr0,
            tc.tile_pool(name="pr1", bufs=1, space="PSUM") as pr1,
            tc.tile_pool(name="pr2", bufs=1, space="PSUM") as pr2,
            tc.tile_pool(name="px0", bufs=1, space="PSUM") as px0,
            tc.tile_pool(name="px1", bufs=1, space="PSUM") as px1,
            tc.tile_pool(name="px2", bufs=1, space="PSUM") as px2,
            tc.tile_pool(name="pp", bufs=2, space="PSUM") as ppool,
        ):
            prp = [pr0, pr1, pr2]
            pxp = [px0, px1, px2]

            # ---- constants + state ----
            ones_t = wpool.tile([128, BC], f16, tag="ones")
            nc.sync.dma_start(ones_t[:], ones[:])
            ident_t = wpool.tile([128, 128], f16, tag="ident")
            nc.sync.dma_start(ident_t[:], ident[:])
            hF = []
            hT = []  # rings of 2 per layer
            for j in range(3):
                f = spool.tile([128, SEC], f32, tag=f"hF{j}")
                nc.sync.dma_start(f[:], h0f[:, j * SEC : (j + 1) * SEC])
                hF.append(f)
                ring = []
                for p in range(2):
                    t = spool.tile([128, SEC], f16, tag=f"hT{j}_{p}")
                    ring.append(t)
                hT.append(ring)
            for j in range(3):
                nc.vector.tensor_copy(hT[j][1][:], hF[j][:])  # h(-1) parity 1

            # ---- weights: 2 large DMAs each, first-use order ----
            def load_w(dram, ncols, tag, parts=2):
                t = wpool.tile([128, ncols], f16, tag=tag)
                step = (ncols + parts - 1) // parts
                for c in range(0, ncols, step):
                    e = min(c + step, ncols)
                    nc.sync.dma_start(t[:, c:e], dram[:, c:e])
                return t

            wh_t = [load_w(wh_d[0], KU * G, "wh0")]
            xall = wpool.tile([128, n_warm * MD * BC], f16, tag="xall")
            nc.sync.dma_start(xall[:], xt[:])
            nshare = max(wx0_c, weff_c)
            wshare = wpool.tile([128, nshare * G], f16, tag="wx0weff")
            nc.sync.dma_start(wshare[:, 0 : wx0_c * G], wx0[:, 0 : wx0_c * G])
            if weff_c > wx0_c:  # weff tail never collides with warm reads
                nc.sync.dma_start(
                    wshare[:, wx0_c * G : weff_c * G],
                    weff[:, wx0_c * G : weff_c * G],
                )
            wh_t.append(load_w(wh_d[1], KU * G, "wh1"))
            wx_t = [wshare, load_w(wx_d[1], KU * G, "wx1")]
            wh_t.append(load_w(wh_d[2], KU * G, "wh2"))
            wx_t.append(load_w(wx_d[2], KU * G, "wx2"))
            wd_t = load_w(wd_d, KU * D, "wd", parts=1)

            # ---- stream emitters ----
            # ZR tile: [z | r]; XH tile: [xh | hh] (closed by the id-add).
            R_cur = {}
            ZXH_cur = {}
            first = {}

            def new_tiles(j):
                R_cur[j] = prp[j].tile(
                    [128, 2 * SEC], f32, tag="zr", name=f"zr{j}"
                )
                ZXH_cur[j] = pxp[j].tile(
                    [128, 2 * SEC], f32, tag="xh", name=f"xh{j}"
                )
                first[j] = {"r": True, "zxh": True}

            def _mm(j, tile_kind, P, col, w_t, wcol, rhs, stop):
                nc.tensor.matmul(
                    P[:, col : col + BC],
                    w_t[:, wcol : wcol + 128],
                    rhs,
                    start=first[j][tile_kind],
                    stop=stop,
                    skip_group_check=True,
                )
                first[j][tile_kind] = False

            def gh_zr(j, t):
                """gh r then z sections from h_j(t-1) into the [z|r] tile."""
                src = hT[j][(t - 1) % 2]
                for m in range(6, 12):
                    c = m % 6
                    for k in range(KU):
                        _mm(j, "r", R_cur[j], SEC + c * BC,
                            wh_t[j], k * G + m * 128,
                            src[:, k * BC : (k + 1) * BC], False)
                for m in range(6):
                    for k in range(KU):
                        _mm(j, "r", R_cur[j], m * BC,
                            wh_t[j], k * G + m * 128,
                            src[:, k * BC : (k + 1) * BC], False)

            def gh_hh(j, t):
                src = hT[j][(t - 1) % 2]
                for m in range(12, 18):
                    c = m % 6
                    for k in range(KU):
                        _mm(j, "zxh", ZXH_cur[j], SEC + c * BC,
                            wh_t[j], k * G + m * 128,
                            src[:, k * BC : (k + 1) * BC], False)

            def gx_zr(j, w_t, kc, rhs_fn, closer=True):
                """gx r then z sections; the last z matmul closes [z|r]."""
                n = 0
                for m in list(range(6, 12)) + list(range(6)):
                    off = SEC if m >= 6 else 0
                    c = m % 6
                    for k in range(kc):
                        n += 1
                        _mm(j, "r", R_cur[j], off + c * BC,
                            w_t, k * G + m * 128, rhs_fn(k),
                            closer and n == 12 * kc)

            def gx_xh(j, w_t, kc, rhs_fn):
                """gx candidate section; the t1 id-accumulate closes [xh|hh]."""
                for m in range(12, 18):
                    c = m % 6
                    for k in range(kc):
                        _mm(j, "zxh", ZXH_cur[j], c * BC,
                            w_t, k * G + m * 128, rhs_fn(k), False)

            def rhs_x(s):
                def rhs(k, _s=s):
                    return xall[:, (_s * MD + k) * BC : (_s * MD + k + 1) * BC]
                return rhs

            def rhs_h(j, t):
                def rhs(k, _t=t):
                    if k >= KU:
                        return ones_t[:, 0:BC]
                    return hT[j][_t % 2][:, k * BC : (k + 1) * BC]
                return rhs

            # ---- gate math ----
            def math_p1(j):
                """sigma(r) only: the chain head."""
                rs = gm.tile([128, SEC], f32, tag=f"rs{j}")
                nc.scalar.activation(rs[:], R_cur[j][:, SEC : 2 * SEC], sig)
                return rs

            def math_p2(j, rs):
                """t1 = r*hh in fp16, accumulated into the xh PSUM region by
                an identity matmul — no DVE add, and this closes [xh|hh]."""
                P = ZXH_cur[j]
                t1 = gm.tile([128, SEC], f16, tag=f"t1{j}")
                nc.vector.tensor_mul(t1[:], rs[:], P[:, SEC : 2 * SEC])
                nc.tensor.matmul(
                    P[:, 0:SEC], ident_t[:], t1[:],
                    start=False, stop=True, skip_group_check=True,
                )

            def math_p3(j, ar=False):
                """sigma(z) + products (DVE, overlapped with the id-add/tanh)."""
                zs = gm.tile([128, SEC], f32, tag=f"zs{j}")
                nc.scalar.activation(zs[:], R_cur[j][:, 0:SEC], sig)
                zh16 = None
                if ar:
                    zh16 = gm.tile([128, SEC], f16, tag=f"zh16{j}")
                    nc.vector.tensor_mul(zh16[:], zs[:], hF[j][:])
                omz = gm.tile([128, SEC], f32, tag=f"omz{j}")
                nc.vector.tensor_scalar(
                    omz[:], zs[:], -1.0, 1.0, alu.mult, alu.add
                )
                zh = gm.tile([128, SEC], f32, tag=f"zh{j}")
                nc.vector.tensor_mul(zh[:], zs[:], hF[j][:])
                return omz, zh, zh16

            def math_p4a(j):
                """tanh straight from PSUM; q in fp16 (a matmul rhs in AR)."""
                P = ZXH_cur[j]
                hc = gm.tile([128, SEC], f32, tag=f"hc{j}")
                nc.scalar.activation(hc[:], P[:, 0:SEC], tanh)
                q = gm.tile([128, SEC], f16, tag=f"q{j}")
                nc.vector.tensor_mul(q[:], omz_of[j][:], hc[:])
                return q

            omz_of = {}

            def math_p4b(j, t, zh, q):
                nc.vector.tensor_add(hT[j][t % 2][:], zh[:], q[:])
                nc.gpsimd.tensor_add(hF[j][:], zh[:], q[:])

            def math_full(j, t):
                rs = math_p1(j)
                math_p2(j, rs)
                omz, zh, _ = math_p3(j)
                omz_of[j] = omz
                q = math_p4a(j)
                math_p4b(j, t, zh, q)

            # ---- dense readout (off the critical chain) ----
            prs_state = {}

            def dense_mm(t):
                Pp = ppool.tile([128, MD * BC], f32, tag="pred", name="pred")
                src = hT[2][(t - 1) % 2]
                n = 0
                for k in range(KU):
                    for m in range(MD):
                        n += 1
                        nc.tensor.matmul(
                            Pp[:, m * BC : (m + 1) * BC],
                            wd_t[:, k * D + m * 128 : k * D + (m + 1) * 128],
                            src[:, k * BC : (k + 1) * BC],
                            start=n == 1,
                            stop=n == KU * MD,
                            skip_group_check=True,
                        )
                prs_state["Pp"] = Pp

            def dense_out(t):
                s = t - n_warm
                Pp = prs_state.pop("Pp")
                sl = s % 8
                if sl == 0:
                    prs_state["buf"] = prpool.tile(
                        [128, 8 * MD * BC], f32, tag="prs", name="prs"
                    )
                prs = prs_state["buf"]
                nc.scalar.copy(
                    prs[:, sl * MD * BC : (sl + 1) * MD * BC], Pp[:]
                )
                if sl == 7 or s == n_ar:
                    grp = s // 8
                    nc.sync.dma_start(
                        out[:, grp * 8 * MD * BC : (grp * 8 + sl + 1) * MD * BC],
                        prs[:, 0 : (sl + 1) * MD * BC],
                    )

            # ================= WARM phase: pipelined 3-layer wavefront ======
            # iteration tt runs maths for L0(tt), L1(tt-1), L2(tt-2) on tiles
            # streamed during iteration tt-1; streams for the next maths are
            # emitted right after the h-state they read is produced.
            for j in range(3):
                new_tiles(j)
            # prologue: tiles for (j, 0) from the initial states / x(0)
            gh_zr(0, 0)
            gh_hh(0, 0)
            gx_zr(0, wshare, wx0_c, rhs_x(0))
            gx_xh(0, wshare, wx0_c, rhs_x(0))
            gh_zr(1, 0)
            gh_hh(1, 0)
            gh_zr(2, 0)
            gh_hh(2, 0)

            for tt in range(n_warm + 2):
                for j in range(3):
                    s = tt - j
                    if not (0 <= s < n_warm):
                        continue
                    math_full(j, s)
                    nxt = s + 1
                    if nxt < n_warm:
                        # streams for (j, nxt): gx reads h_{j-1}(nxt), which
                        # block j-1 of THIS iteration produced (or x); gh
                        # reads h_j(s), just produced above.
                        new_tiles(j)
                        if j == 0:
                            rx = rhs_x(nxt)
                            gx_zr(0, wshare, wx0_c, rx)
                            gh_zr(0, nxt)
                            gx_xh(0, wshare, wx0_c, rx)
                            gh_hh(0, nxt)
                        else:
                            rh = rhs_h(j - 1, nxt)
                            gx_zr(j, wx_t[j], KU, rh)
                            gh_zr(j, nxt)
                            gx_xh(j, wx_t[j], KU, rh)
                            gh_hh(j, nxt)
                    if s == 0 and j < 2:
                        # bootstrap: close tiles(j+1, 0) — its gh half was
                        # emitted in the prologue, gx needs h_j(0) from above
                        rh = rhs_h(j, 0)
                        gx_zr(j + 1, wx_t[j + 1], KU, rh)
                        gx_xh(j + 1, wx_t[j + 1], KU, rh)
                    if j == 0 and s == n_warm - 1:
                        # swap Weff over the wx0 chunks (after last warm use)
                        half = (wx0_c * G) // 2
                        nc.sync.dma_start(wshare[:, 0:half], weff[:, 0:half])
                        nc.sync.dma_start(
                            wshare[:, half : wx0_c * G], weff[:, half : wx0_c * G]
                        )

            # ================= AR phase: strictly serial =================
            # Layer handoffs stream W.(zh16) as soon as the z-products exist
            # (pass A, with the bias ones-chunk) and W.(q16) right after the
            # tanh (pass B) — the next layer never waits for the h16 add.
            def rhs_zh16(zz):
                def rhs(k, _z=zz):
                    if k >= KU:
                        return ones_t[:, 0:BC]
                    return _z[:, k * BC : (k + 1) * BC]
                return rhs

            def rhs_q16(qq):
                def rhs(k, _q=qq):
                    return _q[:, k * BC : (k + 1) * BC]
                return rhs

            for t in range(n_warm, n_steps):
                if t == n_warm:
                    # boundary: classic streams from the warm states
                    for j in range(3):
                        new_tiles(j)
                    gh_zr(0, t)
                    gh_hh(0, t)
                    gh_zr(1, t)
                    gh_hh(1, t)
                    gx_zr(0, wshare, weff_c, rhs_h(2, t - 1))
                    gx_xh(0, wshare, weff_c, rhs_h(2, t - 1))
                dense_mm(t)
                if t == n_warm:
                    gh_zr(2, t)
                    gh_hh(2, t)
                for j in range(3):
                    rs = math_p1(j)
                    math_p2(j, rs)
                    omz, zh, zh16 = math_p3(j, ar=True)
                    omz_of[j] = omz
                    # pass A of the next consumer's gx
                    if j < 2:
                        gx_zr(j + 1, wx_t[j + 1], KU, rhs_zh16(zh16),
                              closer=False)
                        gx_xh(j + 1, wx_t[j + 1], KU, rhs_zh16(zh16))
                    elif t + 1 < n_steps:
                        gx_zr(0, wshare, weff_c, rhs_zh16(zh16), closer=False)
                        gx_xh(0, wshare, weff_c, rhs_zh16(zh16))
                    q = math_p4a(j)
                    # pass B closes the next ZR tile; XH closes via its id-add
                    if j < 2:
                        gx_zr(j + 1, wx_t[j + 1], KU, rhs_q16(q))
                        gx_xh(j + 1, wx_t[j + 1], KU, rhs_q16(q))
                    elif t + 1 < n_steps:
                        gx_zr(0, wshare, KU, rhs_q16(q))
                        gx_xh(0, wshare, KU, rhs_q16(q))
                    math_p4b(j, t, zh, q)
                    if j == 0:
                        dense_out(t)
                    if t + 1 < n_steps:
                        if j < 2:
                            new_tiles(j)
                            gh_zr(j, t + 1)
                            gh_hh(j, t + 1)
                # gh2(t+1) + its tile rotation at the next step top
                if t + 1 < n_steps:
                    new_tiles(2)
                    gh_zr(2, t + 1)
                    gh_hh(2, t + 1)

            # final prediction from h2(n_steps-1)
            dense_mm(n_steps)
            dense_out(n_steps)
    nc.finalize()
    return nc


def kernel(**inputs):
    x = np.asarray(inputs["inputs"], np.float32)
    n_warm, n_ar = T_IN, T_OUT - 1
    x = x[:, :n_warm, :]

    mean = np.asarray(inputs["mean"], np.float32)[0]
    std = np.asarray(inputs["std"], np.float32)[0]
    wd_m = np.asarray(inputs["Wd"], np.float32)
    bd = np.asarray(inputs["bd"], np.float32)
    w1 = np.asarray(inputs["Wx0"], np.float32) / std[:, None]
    weff_m = wd_m @ w1
    beff = (bd - mean) @ w1 + np.asarray(inputs["bi0"], np.float32)

    wx0_a = _prep_weight(np.asarray(inputs["Wx0"], np.float32))
    wx0_c = wx0_a.shape[1] // G
    weff_a = _prep_weight(weff_m, beff)
    weff_c = weff_a.shape[1] // G
    wx_a = {
        j: _prep_weight(np.asarray(inputs[f"Wx{j}"], np.float32)) for j in (1, 2)
    }
    wh_a = {
        j: _prep_weight(np.asarray(inputs[f"Wh{j}"], np.float32)) for j in range(3)
    }
    wd_a = _prep_weight(wd_m)  # bd is zero

    # warm inputs: xt[p, t*MD*BC + k*BC + bi] = x[core*BC+bi, t, k*128+p]
    xt_cores = []
    for core in range(NCORES):
        xs = x[core * BC : (core + 1) * BC]  # [BC, T, D]
        xr = xs.reshape(BC, n_warm, MD, 128).transpose(3, 1, 2, 0)
        xt_cores.append(
            np.ascontiguousarray(xr.reshape(128, n_warm * MD * BC)).astype(F16)
        )

    # initial state: h0f[p, j*SEC + k*BC + bi] = h0_j[k*128+p]
    h0_parts = []
    for j in range(3):
        h = np.asarray(inputs[f"h0_{j}"], np.float32).reshape(KU, 128)
        h0_parts.append(np.repeat(h.transpose(1, 0)[:, :, None], BC, axis=2))
    h0f_a = np.concatenate(h0_parts, axis=1).reshape(128, 3 * SEC)
    h0f_a = np.ascontiguousarray(h0f_a).astype(np.float32)

    ones_a = np.zeros((128, BC), np.float32)
    ones_a[0, :] = 1.0
    ones_a = ones_a.astype(F16)
    ident_a = np.eye(128, dtype=F16)

    nc = _build(n_warm, n_ar, wx0_c, weff_c)
    in_maps = []
    for core in range(NCORES):
        in_maps.append(
            {
                "wx0": wx0_a,
                "weff": weff_a,
                "wx1": wx_a[1],
                "wx2": wx_a[2],
                "wh0": wh_a[0],
                "wh1": wh_a[1],
                "wh2": wh_a[2],
                "wd": wd_a,
                "xt": xt_cores[core],
                "h0f": h0f_a,
                "ones": ones_a,
                "ident": ident_a,
            }
        )
    res = run_bass_kernel_spmd(
        nc,
        in_maps,
        core_ids=list(range(NCORES)),
        trace=os.environ.get("GRU_TRACE", "") == "1",
    )
    kernel._last = res
    kernel._last_nc = nc

    n_out = n_ar + 1
    full = np.empty((B, n_out, D), np.float32)
    for core in range(NCORES):
        o = np.asarray(res.results[core]["out"], np.float32)
        o = o.reshape(128, n_out, MD, BC)
        full[core * BC : (core + 1) * BC] = o.transpose(3, 1, 2, 0).reshape(
            BC, n_out, D
        )
    return full


if __name__ == "__main__":
    print("smoke build only")


# revision 51
# speedup vs baseline: 1.0472x; 1.0006x over previous
"""Trainium2 Bass kernel for the 3-layer AR GRU (nn_AR_RNN_GRU).

Strategy
--------
Data-parallel over batch across 8 NeuronCores (batch 8 per core) — batch
elements are fully independent, so sharding adds ZERO numerical error and
needs ZERO communication.  Each core runs the whole 64-warm + 63-AR
recurrence on its batch slice with all weights replicated (fp16 matmuls,
fp32 gate math — numerically identical to the single-core baseline).

The per-core program is latency-dominated, so the layout is built around
the tile-granular dependency tracker:

 * PSUM is split per layer into TWO single-buffered accumulation tiles:
   ZR = [z | r] and XH = [xh | hh].  sigma(z,r) therefore waits only the
   z/r matmuls, not the whole gate stream; each tile's only readers finish
   mid-chain, before the next step's gh stream reopens it (bufs=1 is safe
   and keeps all 6 tiles + 2 dense-readout banks within the 8 PSUM banks).
 * Gate math per layer-step (critical chain in *bold*):
     ACT  *zr = sigmoid([z|r])*            (one op; ACT runs nothing else
     DVE  *t1 = zr.r * XH.hh ; t1 += XH.xh*    between this and tanh)
     DVE  omz = 1 - zr.z ; zh = zr.z * hF      (off-chain, in tanh window)
     ACT  *hc = tanh(t1)*
     DVE  *q = omz*hc ; h16 = zh+q*  -> next matmul input (fp16)
     Pool hF = zh+q                   (fp32 state, off-path)
   Keeping omz/zh on DVE means q and h16 have no cross-engine waits
   except the tanh they truly depend on.
 * Streams are emitted in semaphore-gate order so the in-order PE queue
   never blocks ready work behind a waiting instruction.  The warm phase
   runs a 3-layer wavefront: iteration tt computes L0(tt), L1(tt-1),
   L2(tt-2) from tiles streamed in the previous iteration, so the three
   chains overlap on the engines; the AR phase is the same layer-major
   emission but consumes tiles within the step (strict serial chain).
 * The AR feedback folds dense+normalize into one matrix:
   gx0 = h2 @ (Wd @ (Wx0/std)) + beff (bias via an extra ones K-chunk);
   the real prediction h2 @ Wd streams off-path, staged in SBUF and
   DMA'd out in 8-step groups.
 * All DRAM images are per-partition contiguous; each weight loads with
   two large DMAs.
"""

import os
import sys

import numpy as np

try:
    import concourse.bass as bass  # noqa: F401
except ImportError:  # grading env fallback
    sys.path.insert(0, "/opt/trn_rl_repo")

import concourse.bass as bass
import concourse.mybir as mybir
import concourse.tile as tile
from concourse import bacc
from concourse.bass_utils import run_bass_kernel_spmd

F16 = np.float16

B = 64  # total batch
NCORES = 8
BC = B // NCORES  # per-core batch (8)
D = 512  # data dim
U = 768  # GRU units
G = 3 * U  # gate columns (z|r|h)
KU = U // 128  # 6 K-chunks for a 768-row operand
MD = D // 128  # 4 M-chunks of data columns

SEC = KU * BC  # 48 columns per gate section

T_IN = int(os.environ.get("GRU_TIN", "64"))
T_OUT = int(os.environ.get("GRU_TOUT", "64"))


def _prep_weight(w, bias=None):
    """[K, G] fp32 (+bias [G]) -> per-partition image [128, n_chunks*G] fp16
    (chunk k at cols [k*G:(k+1)*G]; bias as extra chunk, row 0)."""
    k, g = w.shape
    assert k % 128 == 0
    wp = w.reshape(k // 128, 128, g)
    if bias is not None and float(np.abs(bias).max()) > 0.0:
        bc = np.zeros((1, 128, g), np.float32)
        bc[0, 0, :] = bias
        wp = np.concatenate([wp, bc], axis=0)
    return np.ascontiguousarray(wp.transpose(1, 0, 2).reshape(128, -1)).astype(F16)


def _build(n_warm, n_ar, wx0_c, weff_c):
    nc = bacc.Bacc(num_devices=1, name="gru_ar_dp8")
    f32, f16 = mybir.dt.float32, mybir.dt.float16
    n_steps = n_warm + n_ar  # state steps (t = 0 .. n_steps-1)
    n_out = n_ar + 1

    # ---- DRAM I/O (all per-partition contiguous) ----
    wx0 = nc.dram_tensor("wx0", [128, wx0_c * G], f16, kind="ExternalInput")
    weff = nc.dram_tensor("weff", [128, weff_c * G], f16, kind="ExternalInput")
    wx_d = [None] + [
        nc.dram_tensor(f"wx{j}", [128, KU * G], f16, kind="ExternalInput")
        for j in (1, 2)
    ]
    wh_d = [
        nc.dram_tensor(f"wh{j}", [128, KU * G], f16, kind="ExternalInput")
        for j in range(3)
    ]
    wd_d = nc.dram_tensor("wd", [128, KU * D], f16, kind="ExternalInput")
    dbg_on = os.environ.get("GRU_DBG", "") == "1"
    dbg = (
        nc.dram_tensor("dbg", [128, 6 * SEC], f32, kind="ExternalOutput")
        if dbg_on
        else None
    )
    xt = nc.dram_tensor("xt", [128, n_warm * MD * BC], f16, kind="ExternalInput")
    h0f = nc.dram_tensor("h0f", [128, 3 * SEC], f32, kind="ExternalInput")
    ones = nc.dram_tensor("ones", [128, BC], f16, kind="ExternalInput")
    ident = nc.dram_tensor("ident", [128, 128], f16, kind="ExternalInput")
    out = nc.dram_tensor("out", [128, n_out * MD * BC], f32, kind="ExternalOutput")

    sig = mybir.ActivationFunctionType.Sigmoid
    tanh = mybir.ActivationFunctionType.Tanh
    alu = mybir.AluOpType

    with tile.TileContext(nc) as tc:
        with (
            tc.tile_pool(name="wpool", bufs=1) as wpool,
            tc.tile_pool(name="state", bufs=1) as spool,
            tc.tile_pool(name="gm", bufs=2) as gm,
            tc.tile_pool(name="prs", bufs=2) as prpool,
            tc.tile_pool(name="pr0", bufs=1, space="PSUM") as pr0,
            tc.tile_pool(name="pr1", bufs=1, space="PSUM") as pr1,
            tc.tile_pool(name="pr2", bufs=1, space="PSUM") as pr2,
            tc.tile_pool(name="px0", bufs=1, space="PSUM") as px0,
            tc.tile_pool(name="px1", bufs=1, space="PSUM") as px1,
            tc.tile_pool(name="px2", bufs=1, space="PSUM") as px2,
            tc.tile_pool(name="pp", bufs=2, space="PSUM") as ppool,
        ):
            prp = [pr0, pr1, pr2]
            pxp = [px0, px1, px2]

            # ---- constants + state ----
            ones_t = wpool.tile([128, BC], f16, tag="ones")
            nc.sync.dma_start(ones_t[:], ones[:])
            ident_t = wpool.tile([128, 128], f16, tag="ident")
            nc.sync.dma_start(ident_t[:], ident[:])
            hF = []
            hT = []  # rings of 2 per layer
            for j in range(3):
                f = spool.tile([128, SEC], f32, tag=f"hF{j}")
                nc.sync.dma_start(f[:], h0f[:, j * SEC : (j + 1) * SEC])
                hF.append(f)
                ring = []
                for p in range(2):
                    t = spool.tile([128, SEC], f16, tag=f"hT{j}_{p}")
                    ring.append(t)
                hT.append(ring)
            for j in range(3):
                nc.vector.tensor_copy(hT[j][1][:], hF[j][:])  # h(-1) parity 1

            # ---- weights: 2 large DMAs each, first-use order ----
            def load_w(dram, ncols, tag, parts=2):
                t = wpool.tile([128, ncols], f16, tag=tag)
                step = (ncols + parts - 1) // parts
                for c in range(0, ncols, step):
                    e = min(c + step, ncols)
                    nc.sync.dma_start(t[:, c:e], dram[:, c:e])
                return t

            wh_t = [load_w(wh_d[0], KU * G, "wh0")]
            xall = wpool.tile([128, n_warm * MD * BC], f16, tag="xall")
            nc.sync.dma_start(xall[:], xt[:])
            nshare = max(wx0_c, weff_c)
            wshare = wpool.tile([128, nshare * G], f16, tag="wx0weff")
            nc.sync.dma_start(wshare[:, 0 : wx0_c * G], wx0[:, 0 : wx0_c * G])
            if weff_c > wx0_c:  # weff tail never collides with warm reads
                nc.sync.dma_start(
                    wshare[:, wx0_c * G : weff_c * G],
                    weff[:, wx0_c * G : weff_c * G],
                )
            wh_t.append(load_w(wh_d[1], KU * G, "wh1"))
            wx_t = [wshare, load_w(wx_d[1], KU * G, "wx1")]
            wh_t.append(load_w(wh_d[2], KU * G, "wh2"))
            wx_t.append(load_w(wx_d[2], KU * G, "wx2"))
            wd_t = load_w(wd_d, KU * D, "wd", parts=1)

            # ---- stream emitters ----
            # ZR tile: [z | r]; XH tile: [xh | hh] (closed by the id-add).
            R_cur = {}
            ZXH_cur = {}
            first = {}

            def new_tiles(j):
                R_cur[j] = prp[j].tile(
                    [128, 2 * SEC], f32, tag="zr", name=f"zr{j}"
                )
                ZXH_cur[j] = pxp[j].tile(
                    [128, 2 * SEC], f32, tag="xh", name=f"xh{j}"
                )
                first[j] = {"r": True, "zxh": True}

            def _mm(j, tile_kind, P, col, w_t, wcol, rhs, stop):
                nc.tensor.matmul(
                    P[:, col : col + BC],
                    w_t[:, wcol : wcol + 128],
                    rhs,
                    start=first[j][tile_kind],
                    stop=stop,
                    skip_group_check=True,
                )
                first[j][tile_kind] = False

            def gh_zr(j, t):
                """gh r then z sections from h_j(t-1) into the [z|r] tile."""
                src = hT[j][(t - 1) % 2]
                for m in range(6, 12):
                    c = m % 6
                    for k in range(KU):
                        _mm(j, "r", R_cur[j], SEC + c * BC,
                            wh_t[j], k * G + m * 128,
                            src[:, k * BC : (k + 1) * BC], False)
                for m in range(6):
                    for k in range(KU):
                        _mm(j, "r", R_cur[j], m * BC,
                            wh_t[j], k * G + m * 128,
                            src[:, k * BC : (k + 1) * BC], False)

            def gh_hh(j, t):
                src = hT[j][(t - 1) % 2]
                for m in range(12, 18):
                    c = m % 6
                    for k in range(KU):
                        _mm(j, "zxh", ZXH_cur[j], SEC + c * BC,
                            wh_t[j], k * G + m * 128,
                            src[:, k * BC : (k + 1) * BC], False)

            def gx_zr(j, w_t, kc, rhs_fn, closer=True):
                """gx r then z sections; the last z matmul closes [z|r]."""
                n = 0
                for m in list(range(6, 12)) + list(range(6)):
                    off = SEC if m >= 6 else 0
                    c = m % 6
                    for k in range(kc):
                        n += 1
                        _mm(j, "r", R_cur[j], off + c * BC,
                            w_t, k * G + m * 128, rhs_fn(k),
                            closer and n == 12 * kc)

            def gx_xh(j, w_t, kc, rhs_fn):
                """gx candidate section; the t1 id-accumulate closes [xh|hh]."""
                for m in range(12, 18):
                    c = m % 6
                    for k in range(kc):
                        _mm(j, "zxh", ZXH_cur[j], c * BC,
                            w_t, k * G + m * 128, rhs_fn(k), False)

            def rhs_x(s):
                def rhs(k, _s=s):
                    return xall[:, (_s * MD + k) * BC : (_s * MD + k + 1) * BC]
                return rhs

            def rhs_h(j, t):
                def rhs(k, _t=t):
                    if k >= KU:
                        return ones_t[:, 0:BC]
                    return hT[j][_t % 2][:, k * BC : (k + 1) * BC]
                return rhs

            # ---- gate math ----
            def math_p1(j):
                """sigma(r) only: the chain head."""
                rs = gm.tile([128, SEC], f32, tag=f"rs{j}")
                nc.scalar.activation(rs[:], R_cur[j][:, SEC : 2 * SEC], sig)
                return rs

            def math_p2(j, rs):
                """t1 = r*hh in fp16, accumulated into the xh PSUM region by
                an identity matmul — no DVE add, and this closes [xh|hh]."""
                P = ZXH_cur[j]
                t1 = gm.tile([128, SEC], f16, tag=f"t1{j}")
                nc.vector.tensor_mul(t1[:], rs[:], P[:, SEC : 2 * SEC])
                nc.tensor.matmul(
                    P[:, 0:SEC], ident_t[:], t1[:],
                    start=False, stop=True, skip_group_check=True,
                )

            def math_p3(j, ar=False):
                """sigma(z) + products (DVE, overlapped with the id-add/tanh)."""
                zs = gm.tile([128, SEC], f32, tag=f"zs{j}")
                nc.scalar.activation(zs[:], R_cur[j][:, 0:SEC], sig)
                zh16 = None
                if ar:
                    zh16 = gm.tile([128, SEC], f16, tag=f"zh16{j}")
                    nc.vector.tensor_mul(zh16[:], zs[:], hF[j][:])
                omz = gm.tile([128, SEC], f32, tag=f"omz{j}")
                nc.vector.tensor_scalar(
                    omz[:], zs[:], -1.0, 1.0, alu.mult, alu.add
                )
                zh = gm.tile([128, SEC], f32, tag=f"zh{j}")
                nc.vector.tensor_mul(zh[:], zs[:], hF[j][:])
                return omz, zh, zh16

            def math_p4a(j):
                """tanh straight from PSUM; q in fp16 (a matmul rhs in AR)."""
                P = ZXH_cur[j]
                hc = gm.tile([128, SEC], f32, tag=f"hc{j}")
                nc.scalar.activation(hc[:], P[:, 0:SEC], tanh)
                q = gm.tile([128, SEC], f16, tag=f"q{j}")
                nc.vector.tensor_mul(q[:], omz_of[j][:], hc[:])
                return q

            omz_of = {}

            def math_p4b(j, t, zh, q):
                nc.vector.tensor_add(hT[j][t % 2][:], zh[:], q[:])
                nc.gpsimd.tensor_add(hF[j][:], zh[:], q[:])

            def math_full(j, t):
                rs = math_p1(j)
                math_p2(j, rs)
                omz, zh, _ = math_p3(j)
                omz_of[j] = omz
                q = math_p4a(j)
                math_p4b(j, t, zh, q)

            # ---- dense readout (off the critical chain) ----
            prs_state = {}

            def dense_mm(t):
                Pp = ppool.tile([128, MD * BC], f32, tag="pred", name="pred")
                src = hT[2][(t - 1) % 2]
                n = 0
                for k in range(KU):
                    for m in range(MD):
                        n += 1
                        nc.tensor.matmul(
                            Pp[:, m * BC : (m + 1) * BC],
                            wd_t[:, k * D + m * 128 : k * D + (m + 1) * 128],
                            src[:, k * BC : (k + 1) * BC],
                            start=n == 1,
                            stop=n == KU * MD,
                            skip_group_check=True,
                        )
                prs_state["Pp"] = Pp

            def dense_out(t):
                s = t - n_warm
                Pp = prs_state.pop("Pp")
                sl = s % 8
                if sl == 0:
                    prs_state["buf"] = prpool.tile(
                        [128, 8 * MD * BC], f32, tag="prs", name="prs"
                    )
                prs = prs_state["buf"]
                nc.scalar.copy(
                    prs[:, sl * MD * BC : (sl + 1) * MD * BC], Pp[:]
                )
                if sl == 7 or s == n_ar:
                    grp = s // 8
                    nc.sync.dma_start(
                        out[:, grp * 8 * MD * BC : (grp * 8 + sl + 1) * MD * BC],
                        prs[:, 0 : (sl + 1) * MD * BC],
                    )

            # ================= WARM phase: pipelined 3-layer wavefront ======
            # iteration tt runs maths for L0(tt), L1(tt-1), L2(tt-2) on tiles
            # streamed during iteration tt-1; streams for the next maths are
            # emitted right after the h-state they read is produced.
            for j in range(3):
                new_tiles(j)
            # prologue: tiles for (j, 0) from the initial states / x(0)
            gh_zr(0, 0)
            gh_hh(0, 0)
            gx_zr(0, wshare, wx0_c, rhs_x(0))
            gx_xh(0, wshare, wx0_c, rhs_x(0))
            gh_zr(1, 0)
            gh_hh(1, 0)
            gh_zr(2, 0)
            gh_hh(2, 0)

            for tt in range(n_warm + 2):
                for j in range(3):
                    s = tt - j
                    if not (0 <= s < n_warm):
                        continue
                    math_full(j, s)
                    nxt = s + 1
                    if nxt < n_warm:
                        # streams for (j, nxt): gx reads h_{j-1}(nxt), which
                        # block j-1 of THIS iteration produced (or x); gh
                        # reads h_j(s), just produced above.
                        new_tiles(j)
                        if j == 0:
                            rx = rhs_x(nxt)
                            gx_zr(0, wshare, wx0_c, rx)
                            gh_zr(0, nxt)
                            gx_xh(0, wshare, wx0_c, rx)
                            gh_hh(0, nxt)
                        else:
                            rh = rhs_h(j - 1, nxt)
                            gx_zr(j, wx_t[j], KU, rh)
                            gh_zr(j, nxt)
                            gx_xh(j, wx_t[j], KU, rh)
                            gh_hh(j, nxt)
                    if s == 0 and j < 2:
                        # bootstrap: close tiles(j+1, 0) — its gh half was
                        # emitted in the prologue, gx needs h_j(0) from above
                        rh = rhs_h(j, 0)
                        gx_zr(j + 1, wx_t[j + 1], KU, rh)
                        gx_xh(j + 1, wx_t[j + 1], KU, rh)
                    if j == 0 and s == n_warm - 1:
                        # swap Weff over the wx0 chunks (after last warm use)
                        half = (wx0_c * G) // 2
                        nc.sync.dma_start(wshare[:, 0:half], weff[:, 0:half])
                        nc.sync.dma_start(
                            wshare[:, half : wx0_c * G], weff[:, half : wx0_c * G]
                        )

            # ================= AR phase: strictly serial =================
            # Layer handoffs stream W.(zh16) as soon as the z-products exist
            # (pass A, with the bias ones-chunk) and W.(q16) right after the
            # tanh (pass B) — the next layer never waits for the h16 add.
            def rhs_zh16(zz):
                def rhs(k, _z=zz):
                    if k >= KU:
                        return ones_t[:, 0:BC]
                    return _z[:, k * BC : (k + 1) * BC]
                return rhs

            def rhs_q16(qq):
                def rhs(k, _q=qq):
                    return _q[:, k * BC : (k + 1) * BC]
                return rhs

            pending_gh = [None]
            for t in range(n_warm, n_steps):
                if t == n_warm:
                    # boundary: classic streams from the warm states
                    for j in range(3):
                        new_tiles(j)
                    gh_zr(0, t)
                    gh_hh(0, t)
                    gh_zr(1, t)
                    gh_hh(1, t)
                    gx_zr(0, wshare, weff_c, rhs_h(2, t - 1))
                    gx_xh(0, wshare, weff_c, rhs_h(2, t - 1))
                if t == n_warm:
                    dense_mm(t)
                    gh_zr(2, t)
                    gh_hh(2, t)
                for j in range(3):
                    rs = math_p1(j)
                    math_p2(j, rs)
                    # deferred long streams: emitted after this block's id-add
                    # so the short chain-critical matmul never queues behind
                    # a 216-instruction stream on the in-order PE
                    if pending_gh[0] is not None:
                        fn = pending_gh[0]
                        pending_gh[0] = None
                        fn()
                    omz, zh, zh16 = math_p3(j, ar=True)
                    omz_of[j] = omz
                    # pass A of the next consumer's gx
                    if j < 2:
                        gx_zr(j + 1, wx_t[j + 1], KU, rhs_zh16(zh16),
                              closer=False)
                        gx_xh(j + 1, wx_t[j + 1], KU, rhs_zh16(zh16))
                    elif t + 1 < n_steps:
                        gx_zr(0, wshare, weff_c, rhs_zh16(zh16), closer=False)
                        gx_xh(0, wshare, weff_c, rhs_zh16(zh16))
                    q = math_p4a(j)
                    # pass B closes the next ZR tile; XH closes via its id-add
                    if j < 2:
                        gx_zr(j + 1, wx_t[j + 1], KU, rhs_q16(q))
                        gx_xh(j + 1, wx_t[j + 1], KU, rhs_q16(q))
                    elif t + 1 < n_steps:
                        gx_zr(0, wshare, KU, rhs_q16(q))
                        gx_xh(0, wshare, KU, rhs_q16(q))
                    math_p4b(j, t, zh, q)
                    if j == 0:
                        dense_out(t)
                    if t + 1 < n_steps and j < 2:
                        new_tiles(j)

                        def _gh(jj=j, tt=t + 1):
                            gh_zr(jj, tt)
                            gh_hh(jj, tt)

                        pending_gh[0] = _gh
                if t + 1 < n_steps:
                    new_tiles(2)

                    def _gh2(tt=t + 1):
                        dense_mm(tt)
                        gh_zr(2, tt)
                        gh_hh(2, tt)

                    pending_gh[0] = _gh2

            # final prediction from h2(n_steps-1)
            dense_mm(n_steps)
            dense_out(n_steps)
    nc.finalize()
    return nc


def kernel(**inputs):
    x = np.asarray(inputs["inputs"], np.float32)
    n_warm, n_ar = T_IN, T_OUT - 1
    x = x[:, :n_warm, :]

    mean = np.asarray(inputs["mean"], np.float32)[0]
    std = np.asarray(inputs["std"], np.float32)[0]
    wd_m = np.asarray(inputs["Wd"], np.float32)
    bd = np.asarray(inputs["bd"], np.float32)
    w1 = np.asarray(inputs["Wx0"], np.float32) / std[:, None]
    weff_m = wd_m @ w1
    beff = (bd - mean) @ w1 + np.asarray(inputs["bi0"], np.float32)

    wx0_a = _prep_weight(np.asarray(inputs["Wx0"], np.float32))
    wx0_c = wx0_a.shape[1] // G
    weff_a = _prep_weight(weff_m, beff)
    weff_c = weff_a.shape[1] // G
    wx_a = {
        j: _prep_weight(np.asarray(inputs[f"Wx{j}"], np.float32)) for j in (1, 2)
    }
    wh_a = {
        j: _prep_weight(np.asarray(inputs[f"Wh{j}"], np.float32)) for j in range(3)
    }
    wd_a = _prep_weight(wd_m)  # bd is zero

    # warm inputs: xt[p, t*MD*BC + k*BC + bi] = x[core*BC+bi, t, k*128+p]
    xt_cores = []
    for core in range(NCORES):
        xs = x[core * BC : (core + 1) * BC]  # [BC, T, D]
        xr = xs.reshape(BC, n_warm, MD, 128).transpose(3, 1, 2, 0)
        xt_cores.append(
            np.ascontiguousarray(xr.reshape(128, n_warm * MD * BC)).astype(F16)
        )

    # initial state: h0f[p, j*SEC + k*BC + bi] = h0_j[k*128+p]
    h0_parts = []
    for j in range(3):
        h = np.asarray(inputs[f"h0_{j}"], np.float32).reshape(KU, 128)
        h0_parts.append(np.repeat(h.transpose(1, 0)[:, :, None], BC, axis=2))
    h0f_a = np.concatenate(h0_parts, axis=1).reshape(128, 3 * SEC)
    h0f_a = np.ascontiguousarray(h0f_a).astype(np.float32)

    ones_a = np.zeros((128, BC), np.float32)
    ones_a[0, :] = 1.0
    ones_a = ones_a.astype(F16)
    ident_a = np.eye(128, dtype=F16)

    nc = _build(n_warm, n_ar, wx0_c, weff_c)
    in_maps = []
    for core in range(NCORES):
        in_maps.append(
            {
                "wx0": wx0_a,
                "weff": weff_a,
                "wx1": wx_a[1],
                "wx2": wx_a[2],
                "wh0": wh_a[0],
                "wh1": wh_a[1],
                "wh2": wh_a[2],
                "wd": wd_a,
                "xt": xt_cores[core],
                "h0f": h0f_a,
                "ones": ones_a,
                "ident": ident_a,
            }
        )
    res = run_bass_kernel_spmd(
        nc,
        in_maps,
        core_ids=list(range(NCORES)),
        trace=os.environ.get("GRU_TRACE", "") == "1",
    )
    kernel._last = res
    kernel._last_nc = nc

    n_out = n_ar + 1
    full = np.empty((B, n_out, D), np.float32)
    for core in range(NCORES):
        o = np.asarray(res.results[core]["out"], np.float32)
        o = o.reshape(128, n_out, MD, BC)
        full[core * BC : (core + 1) * BC] = o.transpose(3, 1, 2, 0).reshape(
            BC, n_out, D
        )
    return full


if __name__ == "__main__":
    print("smoke build only")


# revision 52
# speedup vs baseline: 1.0519x; 1.0045x over previous
"""Trainium2 Bass kernel for the 3-layer AR GRU (nn_AR_RNN_GRU).

Strategy
--------
Data-parallel over batch across 8 NeuronCores (batch 8 per core) — batch
elements are fully independent, so sharding adds ZERO numerical error and
needs ZERO communication.  Each core runs the whole 64-warm + 63-AR
recurrence on its batch slice with all weights replicated (fp16 matmuls,
fp32 gate math — numerically identical to the single-core baseline).

The per-core program is latency-dominated, so the layout is built around
the tile-granular dependency tracker:

 * PSUM is split per layer into TWO single-buffered accumulation tiles:
   ZR = [z | r] and XH = [xh | hh].  sigma(z,r) therefore waits only the
   z/r matmuls, not the whole gate stream; each tile's only readers finish
   mid-chain, before the next step's gh stream reopens it (bufs=1 is safe
   and keeps all 6 tiles + 2 dense-readout banks within the 8 PSUM banks).
 * Gate math per layer-step (critical chain in *bold*):
     ACT  *zr = sigmoid([z|r])*            (one op; ACT runs nothing else
     DVE  *t1 = zr.r * XH.hh ; t1 += XH.xh*    between this and tanh)
     DVE  omz = 1 - zr.z ; zh = zr.z * hF      (off-chain, in tanh window)
     ACT  *hc = tanh(t1)*
     DVE  *q = omz*hc ; h16 = zh+q*  -> next matmul input (fp16)
     Pool hF = zh+q                   (fp32 state, off-path)
   Keeping omz/zh on DVE means q and h16 have no cross-engine waits
   except the tanh they truly depend on.
 * Streams are emitted in semaphore-gate order so the in-order PE queue
   never blocks ready work behind a waiting instruction.  The warm phase
   runs a 3-layer wavefront: iteration tt computes L0(tt), L1(tt-1),
   L2(tt-2) from tiles streamed in the previous iteration, so the three
   chains overlap on the engines; the AR phase is the same layer-major
   emission but consumes tiles within the step (strict serial chain).
 * The AR feedback folds dense+normalize into one matrix:
   gx0 = h2 @ (Wd @ (Wx0/std)) + beff (bias via an extra ones K-chunk);
   the real prediction h2 @ Wd streams off-path, staged in SBUF and
   DMA'd out in 8-step groups.
 * All DRAM images are per-partition contiguous; each weight loads with
   two large DMAs.
"""

import os
import sys

import numpy as np

try:
    import concourse.bass as bass  # noqa: F401
except ImportError:  # grading env fallback
    sys.path.insert(0, "/opt/trn_rl_repo")

import concourse.bass as bass
import concourse.mybir as mybir
import concourse.tile as tile
from concourse import bacc
from concourse.bass_utils import run_bass_kernel_spmd

F16 = np.float16

B = 64  # total batch
NCORES = 8
BC = B // NCORES  # per-core batch (8)
D = 512  # data dim
U = 768  # GRU units
G = 3 * U  # gate columns (z|r|h)
KU = U // 128  # 6 K-chunks for a 768-row operand
MD = D // 128  # 4 M-chunks of data columns

SEC = KU * BC  # 48 columns per gate section

T_IN = int(os.environ.get("GRU_TIN", "64"))
T_OUT = int(os.environ.get("GRU_TOUT", "64"))


def _prep_weight(w, bias=None):
    """[K, G] fp32 (+bias [G]) -> per-partition image [128, n_chunks*G] fp16
    (chunk k at cols [k*G:(k+1)*G]; bias as extra chunk, row 0)."""
    k, g = w.shape
    assert k % 128 == 0
    wp = w.reshape(k // 128, 128, g)
    if bias is not None and float(np.abs(bias).max()) > 0.0:
        bc = np.zeros((1, 128, g), np.float32)
        bc[0, 0, :] = bias
        wp = np.concatenate([wp, bc], axis=0)
    return np.ascontiguousarray(wp.transpose(1, 0, 2).reshape(128, -1)).astype(F16)


def _build(n_warm, n_ar, wx0_c, weff_c):
    nc = bacc.Bacc(num_devices=1, name="gru_ar_dp8")
    f32, f16 = mybir.dt.float32, mybir.dt.float16
    n_steps = n_warm + n_ar  # state steps (t = 0 .. n_steps-1)
    n_out = n_ar + 1

    # ---- DRAM I/O (all per-partition contiguous) ----
    wx0 = nc.dram_tensor("wx0", [128, wx0_c * G], f16, kind="ExternalInput")
    weff = nc.dram_tensor("weff", [128, weff_c * G], f16, kind="ExternalInput")
    wx_d = [None] + [
        nc.dram_tensor(f"wx{j}", [128, KU * G], f16, kind="ExternalInput")
        for j in (1, 2)
    ]
    wh_d = [
        nc.dram_tensor(f"wh{j}", [128, KU * G], f16, kind="ExternalInput")
        for j in range(3)
    ]
    wd_d = nc.dram_tensor("wd", [128, KU * D], f16, kind="ExternalInput")
    dbg_on = os.environ.get("GRU_DBG", "") == "1"
    dbg = (
        nc.dram_tensor("dbg", [128, 6 * SEC], f32, kind="ExternalOutput")
        if dbg_on
        else None
    )
    xt = nc.dram_tensor("xt", [128, n_warm * MD * BC], f16, kind="ExternalInput")
    h0f = nc.dram_tensor("h0f", [128, 3 * SEC], f32, kind="ExternalInput")
    ones = nc.dram_tensor("ones", [128, BC], f16, kind="ExternalInput")
    ident = nc.dram_tensor("ident", [128, 128], f16, kind="ExternalInput")
    out = nc.dram_tensor("out", [128, n_out * MD * BC], f32, kind="ExternalOutput")

    sig = mybir.ActivationFunctionType.Sigmoid
    tanh = mybir.ActivationFunctionType.Tanh
    alu = mybir.AluOpType

    with tile.TileContext(nc) as tc:
        with (
            tc.tile_pool(name="wpool", bufs=1) as wpool,
            tc.tile_pool(name="state", bufs=1) as spool,
            tc.tile_pool(name="gm", bufs=2) as gm,
            tc.tile_pool(name="prs", bufs=2) as prpool,
            tc.tile_pool(name="pr0", bufs=1, space="PSUM") as pr0,
            tc.tile_pool(name="pr1", bufs=1, space="PSUM") as pr1,
            tc.tile_pool(name="pr2", bufs=1, space="PSUM") as pr2,
            tc.tile_pool(name="px0", bufs=1, space="PSUM") as px0,
            tc.tile_pool(name="px1", bufs=1, space="PSUM") as px1,
            tc.tile_pool(name="px2", bufs=1, space="PSUM") as px2,
            tc.tile_pool(name="pp", bufs=2, space="PSUM") as ppool,
        ):
            prp = [pr0, pr1, pr2]
            pxp = [px0, px1, px2]

            # ---- constants + state ----
            ones_t = wpool.tile([128, BC], f16, tag="ones")
            nc.sync.dma_start(ones_t[:], ones[:])
            ident_t = wpool.tile([128, 128], f16, tag="ident")
            nc.sync.dma_start(ident_t[:], ident[:])
            hF = []
            hT = []  # rings of 2 per layer
            for j in range(3):
                f = spool.tile([128, SEC], f32, tag=f"hF{j}")
                nc.sync.dma_start(f[:], h0f[:, j * SEC : (j + 1) * SEC])
                hF.append(f)
                ring = []
                for p in range(2):
                    t = spool.tile([128, SEC], f16, tag=f"hT{j}_{p}")
                    ring.append(t)
                hT.append(ring)
            for j in range(3):
                nc.vector.tensor_copy(hT[j][1][:], hF[j][:])  # h(-1) parity 1

            # ---- weights: 2 large DMAs each, first-use order ----
            def load_w(dram, ncols, tag, parts=2):
                t = wpool.tile([128, ncols], f16, tag=tag)
                step = (ncols + parts - 1) // parts
                for c in range(0, ncols, step):
                    e = min(c + step, ncols)
                    nc.sync.dma_start(t[:, c:e], dram[:, c:e])
                return t

            wh_t = [load_w(wh_d[0], KU * G, "wh0")]
            xall = wpool.tile([128, n_warm * MD * BC], f16, tag="xall")
            nc.sync.dma_start(xall[:], xt[:])
            nshare = max(wx0_c, weff_c)
            wshare = wpool.tile([128, nshare * G], f16, tag="wx0weff")
            nc.sync.dma_start(wshare[:, 0 : wx0_c * G], wx0[:, 0 : wx0_c * G])
            if weff_c > wx0_c:  # weff tail never collides with warm reads
                nc.sync.dma_start(
                    wshare[:, wx0_c * G : weff_c * G],
                    weff[:, wx0_c * G : weff_c * G],
                )
            wh_t.append(load_w(wh_d[1], KU * G, "wh1"))
            wx_t = [wshare, load_w(wx_d[1], KU * G, "wx1")]
            wh_t.append(load_w(wh_d[2], KU * G, "wh2"))
            wx_t.append(load_w(wx_d[2], KU * G, "wx2"))
            wd_t = load_w(wd_d, KU * D, "wd", parts=1)

            # ---- stream emitters ----
            # ZR tile: [z | r]; XH tile: [xh | hh] (closed by the id-add).
            R_cur = {}
            ZXH_cur = {}
            first = {}

            def new_tiles(j):
                R_cur[j] = prp[j].tile(
                    [128, 2 * SEC], f32, tag="zr", name=f"zr{j}"
                )
                ZXH_cur[j] = pxp[j].tile(
                    [128, 2 * SEC], f32, tag="xh", name=f"xh{j}"
                )
                first[j] = {"r": True, "zxh": True}

            def _mm(j, tile_kind, P, col, w_t, wcol, rhs, stop):
                nc.tensor.matmul(
                    P[:, col : col + BC],
                    w_t[:, wcol : wcol + 128],
                    rhs,
                    start=first[j][tile_kind],
                    stop=stop,
                    skip_group_check=True,
                )
                first[j][tile_kind] = False

            def gh_zr(j, t):
                """gh r then z sections from h_j(t-1) into the [z|r] tile."""
                src = hT[j][(t - 1) % 2]
                for m in range(6, 12):
                    c = m % 6
                    for k in range(KU):
                        _mm(j, "r", R_cur[j], SEC + c * BC,
                            wh_t[j], k * G + m * 128,
                            src[:, k * BC : (k + 1) * BC], False)
                for m in range(6):
                    for k in range(KU):
                        _mm(j, "r", R_cur[j], m * BC,
                            wh_t[j], k * G + m * 128,
                            src[:, k * BC : (k + 1) * BC], False)

            def gh_hh(j, t):
                src = hT[j][(t - 1) % 2]
                for m in range(12, 18):
                    c = m % 6
                    for k in range(KU):
                        _mm(j, "zxh", ZXH_cur[j], SEC + c * BC,
                            wh_t[j], k * G + m * 128,
                            src[:, k * BC : (k + 1) * BC], False)

            def gx_zr(j, w_t, kc, rhs_fn, closer=True):
                """gx r then z sections; the last z matmul closes [z|r]."""
                n = 0
                for m in list(range(6, 12)) + list(range(6)):
                    off = SEC if m >= 6 else 0
                    c = m % 6
                    for k in range(kc):
                        n += 1
                        _mm(j, "r", R_cur[j], off + c * BC,
                            w_t, k * G + m * 128, rhs_fn(k),
                            closer and n == 12 * kc)

            def gx_xh(j, w_t, kc, rhs_fn):
                """gx candidate section; the t1 id-accumulate closes [xh|hh]."""
                for m in range(12, 18):
                    c = m % 6
                    for k in range(kc):
                        _mm(j, "zxh", ZXH_cur[j], c * BC,
                            w_t, k * G + m * 128, rhs_fn(k), False)

            def rhs_x(s):
                def rhs(k, _s=s):
                    return xall[:, (_s * MD + k) * BC : (_s * MD + k + 1) * BC]
                return rhs

            def rhs_h(j, t):
                def rhs(k, _t=t):
                    if k >= KU:
                        return ones_t[:, 0:BC]
                    return hT[j][_t % 2][:, k * BC : (k + 1) * BC]
                return rhs

            # ---- gate math ----
            def math_p1(j):
                """sigma(r) only: the chain head."""
                rs = gm.tile([128, SEC], f32, tag=f"rs{j}")
                nc.scalar.activation(rs[:], R_cur[j][:, SEC : 2 * SEC], sig)
                return rs

            def math_p2(j, rs):
                """t1 = r*hh in fp16, accumulated into the xh PSUM region by
                an identity matmul — no DVE add, and this closes [xh|hh]."""
                P = ZXH_cur[j]
                t1 = gm.tile([128, SEC], f16, tag=f"t1{j}")
                nc.vector.tensor_mul(t1[:], rs[:], P[:, SEC : 2 * SEC])
                nc.tensor.matmul(
                    P[:, 0:SEC], ident_t[:], t1[:],
                    start=False, stop=True, skip_group_check=True,
                )

            def math_p3(j, ar=False):
                """sigma(z) + products (DVE, overlapped with the id-add/tanh)."""
                zs = gm.tile([128, SEC], f32, tag=f"zs{j}")
                nc.scalar.activation(zs[:], R_cur[j][:, 0:SEC], sig)
                zh16 = None
                if ar:
                    zh16 = gm.tile([128, SEC], f16, tag=f"zh16{j}")
                    nc.vector.tensor_mul(zh16[:], zs[:], hF[j][:])
                omz = gm.tile([128, SEC], f32, tag=f"omz{j}")
                nc.vector.tensor_scalar(
                    omz[:], zs[:], -1.0, 1.0, alu.mult, alu.add
                )
                zh = gm.tile([128, SEC], f32, tag=f"zh{j}")
                nc.vector.tensor_mul(zh[:], zs[:], hF[j][:])
                return omz, zh, zh16

            def math_p4a(j):
                """tanh straight from PSUM; q in fp16 (a matmul rhs in AR)."""
                P = ZXH_cur[j]
                hc = gm.tile([128, SEC], f32, tag=f"hc{j}")
                nc.scalar.activation(hc[:], P[:, 0:SEC], tanh)
                q = gm.tile([128, SEC], f16, tag=f"q{j}")
                nc.vector.tensor_mul(q[:], omz_of[j][:], hc[:])
                return q

            omz_of = {}

            def math_p4b(j, t, zh, q):
                nc.vector.tensor_add(hT[j][t % 2][:], zh[:], q[:])
                nc.vector.tensor_add(hF[j][:], zh[:], q[:])

            def math_full(j, t):
                rs = math_p1(j)
                math_p2(j, rs)
                omz, zh, _ = math_p3(j)
                omz_of[j] = omz
                q = math_p4a(j)
                math_p4b(j, t, zh, q)

            # ---- dense readout (off the critical chain) ----
            prs_state = {}

            def dense_mm(t):
                Pp = ppool.tile([128, MD * BC], f32, tag="pred", name="pred")
                src = hT[2][(t - 1) % 2]
                n = 0
                for k in range(KU):
                    for m in range(MD):
                        n += 1
                        nc.tensor.matmul(
                            Pp[:, m * BC : (m + 1) * BC],
                            wd_t[:, k * D + m * 128 : k * D + (m + 1) * 128],
                            src[:, k * BC : (k + 1) * BC],
                            start=n == 1,
                            stop=n == KU * MD,
                            skip_group_check=True,
                        )
                prs_state["Pp"] = Pp

            def dense_out(t):
                s = t - n_warm
                Pp = prs_state.pop("Pp")
                sl = s % 8
                if sl == 0:
                    prs_state["buf"] = prpool.tile(
                        [128, 8 * MD * BC], f32, tag="prs", name="prs"
                    )
                prs = prs_state["buf"]
                nc.scalar.copy(
                    prs[:, sl * MD * BC : (sl + 1) * MD * BC], Pp[:]
                )
                if sl == 7 or s == n_ar:
                    grp = s // 8
                    nc.sync.dma_start(
                        out[:, grp * 8 * MD * BC : (grp * 8 + sl + 1) * MD * BC],
                        prs[:, 0 : (sl + 1) * MD * BC],
                    )

            # ================= WARM phase: pipelined 3-layer wavefront ======
            # iteration tt runs maths for L0(tt), L1(tt-1), L2(tt-2) on tiles
            # streamed during iteration tt-1; streams for the next maths are
            # emitted right after the h-state they read is produced.
            for j in range(3):
                new_tiles(j)
            # prologue: tiles for (j, 0) from the initial states / x(0)
            gh_zr(0, 0)
            gh_hh(0, 0)
            gx_zr(0, wshare, wx0_c, rhs_x(0))
            gx_xh(0, wshare, wx0_c, rhs_x(0))
            gh_zr(1, 0)
            gh_hh(1, 0)
            gh_zr(2, 0)
            gh_hh(2, 0)

            for tt in range(n_warm + 2):
                for j in range(3):
                    s = tt - j
                    if not (0 <= s < n_warm):
                        continue
                    math_full(j, s)
                    nxt = s + 1
                    if nxt < n_warm:
                        # streams for (j, nxt): gx reads h_{j-1}(nxt), which
                        # block j-1 of THIS iteration produced (or x); gh
                        # reads h_j(s), just produced above.
                        new_tiles(j)
                        if j == 0:
                            rx = rhs_x(nxt)
                            gx_zr(0, wshare, wx0_c, rx)
                            gh_zr(0, nxt)
                            gx_xh(0, wshare, wx0_c, rx)
                            gh_hh(0, nxt)
                        else:
                            rh = rhs_h(j - 1, nxt)
                            gx_zr(j, wx_t[j], KU, rh)
                            gh_zr(j, nxt)
                            gx_xh(j, wx_t[j], KU, rh)
                            gh_hh(j, nxt)
                    if s == 0 and j < 2:
                        # bootstrap: close tiles(j+1, 0) — its gh half was
                        # emitted in the prologue, gx needs h_j(0) from above
                        rh = rhs_h(j, 0)
                        gx_zr(j + 1, wx_t[j + 1], KU, rh)
                        gx_xh(j + 1, wx_t[j + 1], KU, rh)
                    if j == 0 and s == n_warm - 1:
                        # swap Weff over the wx0 chunks (after last warm use)
                        half = (wx0_c * G) // 2
                        nc.sync.dma_start(wshare[:, 0:half], weff[:, 0:half])
                        nc.sync.dma_start(
                            wshare[:, half : wx0_c * G], weff[:, half : wx0_c * G]
                        )

            # ================= AR phase: strictly serial =================
            # Layer handoffs stream W.(zh16) as soon as the z-products exist
            # (pass A, with the bias ones-chunk) and W.(q16) right after the
            # tanh (pass B) — the next layer never waits for the h16 add.
            def rhs_zh16(zz):
                def rhs(k, _z=zz):
                    if k >= KU:
                        return ones_t[:, 0:BC]
                    return _z[:, k * BC : (k + 1) * BC]
                return rhs

            def rhs_q16(qq):
                def rhs(k, _q=qq):
                    return _q[:, k * BC : (k + 1) * BC]
                return rhs

            pending_gh = [None]
            for t in range(n_warm, n_steps):
                if t == n_warm:
                    # boundary: classic streams from the warm states
                    for j in range(3):
                        new_tiles(j)
                    gh_zr(0, t)
                    gh_hh(0, t)
                    gh_zr(1, t)
                    gh_hh(1, t)
                    gx_zr(0, wshare, weff_c, rhs_h(2, t - 1))
                    gx_xh(0, wshare, weff_c, rhs_h(2, t - 1))
                if t == n_warm:
                    dense_mm(t)
                    gh_zr(2, t)
                    gh_hh(2, t)
                for j in range(3):
                    rs = math_p1(j)
                    math_p2(j, rs)
                    # deferred long streams: emitted after this block's id-add
                    # so the short chain-critical matmul never queues behind
                    # a 216-instruction stream on the in-order PE
                    if pending_gh[0] is not None:
                        fn = pending_gh[0]
                        pending_gh[0] = None
                        fn()
                    omz, zh, zh16 = math_p3(j, ar=True)
                    omz_of[j] = omz
                    # pass A of the next consumer's gx
                    if j < 2:
                        gx_zr(j + 1, wx_t[j + 1], KU, rhs_zh16(zh16),
                              closer=False)
                        gx_xh(j + 1, wx_t[j + 1], KU, rhs_zh16(zh16))
                    elif t + 1 < n_steps:
                        gx_zr(0, wshare, weff_c, rhs_zh16(zh16), closer=False)
                        gx_xh(0, wshare, weff_c, rhs_zh16(zh16))
                    q = math_p4a(j)
                    # pass B closes the next ZR tile; XH closes via its id-add
                    if j < 2:
                        gx_zr(j + 1, wx_t[j + 1], KU, rhs_q16(q))
                        gx_xh(j + 1, wx_t[j + 1], KU, rhs_q16(q))
                    elif t + 1 < n_steps:
                        gx_zr(0, wshare, KU, rhs_q16(q))
                        gx_xh(0, wshare, KU, rhs_q16(q))
                    math_p4b(j, t, zh, q)
                    if j == 0:
                        dense_out(t)
                    if t + 1 < n_steps and j < 2:
                        new_tiles(j)

                        def _gh(jj=j, tt=t + 1):
                            gh_zr(jj, tt)
                            gh_hh(jj, tt)

                        pending_gh[0] = _gh
                if t + 1 < n_steps:
                    new_tiles(2)

                    def _gh2(tt=t + 1):
                        dense_mm(tt)
                        gh_zr(2, tt)
                        gh_hh(2, tt)

                    pending_gh[0] = _gh2

            # final prediction from h2(n_steps-1)
            dense_mm(n_steps)
            dense_out(n_steps)
    nc.finalize()
    return nc


def kernel(**inputs):
    x = np.asarray(inputs["inputs"], np.float32)
    n_warm, n_ar = T_IN, T_OUT - 1
    x = x[:, :n_warm, :]

    mean = np.asarray(inputs["mean"], np.float32)[0]
    std = np.asarray(inputs["std"], np.float32)[0]
    wd_m = np.asarray(inputs["Wd"], np.float32)
    bd = np.asarray(inputs["bd"], np.float32)
    w1 = np.asarray(inputs["Wx0"], np.float32) / std[:, None]
    weff_m = wd_m @ w1
    beff = (bd - mean) @ w1 + np.asarray(inputs["bi0"], np.float32)

    wx0_a = _prep_weight(np.asarray(inputs["Wx0"], np.float32))
    wx0_c = wx0_a.shape[1] // G
    weff_a = _prep_weight(weff_m, beff)
    weff_c = weff_a.shape[1] // G
    wx_a = {
        j: _prep_weight(np.asarray(inputs[f"Wx{j}"], np.float32)) for j in (1, 2)
    }
    wh_a = {
        j: _prep_weight(np.asarray(inputs[f"Wh{j}"], np.float32)) for j in range(3)
    }
    wd_a = _prep_weight(wd_m)  # bd is zero

    # warm inputs: xt[p, t*MD*BC + k*BC + bi] = x[core*BC+bi, t, k*128+p]
    xt_cores = []
    for core in range(NCORES):
        xs = x[core * BC : (core + 1) * BC]  # [BC, T, D]
        xr = xs.reshape(BC, n_warm, MD, 128).transpose(3, 1, 2, 0)
        xt_cores.append(
            np.ascontiguousarray(xr.reshape(128, n_warm * MD * BC)).astype(F16)
        )

    # initial state: h0f[p, j*SEC + k*BC + bi] = h0_j[k*128+p]
    h0_parts = []
    for j in range(3):
        h = np.asarray(inputs[f"h0_{j}"], np.float32).reshape(KU, 128)
        h0_parts.append(np.repeat(h.transpose(1, 0)[:, :, None], BC, axis=2))
    h0f_a = np.concatenate(h0_parts, axis=1).reshape(128, 3 * SEC)
    h0f_a = np.ascontiguousarray(h0f_a).astype(np.float32)

    ones_a = np.zeros((128, BC), np.float32)
    ones_a[0, :] = 1.0
    ones_a = ones_a.astype(F16)
    ident_a = np.eye(128, dtype=F16)

    nc = _build(n_warm, n_ar, wx0_c, weff_c)
    in_maps = []
    for core in range(NCORES):
        in_maps.append(
            {
                "wx0": wx0_a,
                "weff": weff_a,
                "wx1": wx_a[1],
                "wx2": wx_a[2],
                "wh0": wh_a[0],
                "wh1": wh_a[1],
                "wh2": wh_a[2],
                "wd": wd_a,
                "xt": xt_cores[core],
                "h0f": h0f_a,
                "ones": ones_a,
                "ident": ident_a,
            }
        )
    res = run_bass_kernel_spmd(
        nc,
        in_maps,
        core_ids=list(range(NCORES)),
        trace=os.environ.get("GRU_TRACE", "") == "1",
    )
    kernel._last = res
    kernel._last_nc = nc

    n_out = n_ar + 1
    full = np.empty((B, n_out, D), np.float32)
    for core in range(NCORES):
        o = np.asarray(res.results[core]["out"], np.float32)
        o = o.reshape(128, n_out, MD, BC)
        full[core * BC : (core + 1) * BC] = o.transpose(3, 1, 2, 0).reshape(
            BC, n_out, D
        )
    return full


if __name__ == "__main__":
    print("smoke build only")
